# revision 10
# baseline (speedup 1.0000x reference)
"""Trainium2 Bass kernel for BondingGraphGNN (gnn_message_passing).

Model (see reference):
  h = relu(x @ W_emb)
  4x: m = h @ W_msg[i]; agg = scatter_add(m[src] -> dst); h = GRU(agg, h)
  h = relu(h); pooled = segment_mean(h, batch); out = softplus(relu(pooled@W1+b1)@W2+b2)

Distribution: graph-aligned node sharding across 8 cores. Per step each core
computes messages for its nodes (bf16), AllGathers the full node-message
table to DRAM, then aggregates incoming edges with a RANK-GATHER layout:
nodes are permuted so each 512-node chunk has near-uniform in-degree per
table half; the r-th incoming edge of every slot is fetched in one
transposed dma_gather block [feat, 512], and rank blocks are summed by
identity-matmul accumulation in PSUM (f32). Slots with fewer edges point at
a guaranteed-zero table row. No per-edge one-hot matrices are needed.

Host side does only data layout (shard/sort/pad/index building) - all float
math runs on device.
"""

import os
import numpy as np

# the trimmed axon package in some containers lacks the NTFF profile hook
# module; stub it so run_bass_kernel_spmd(trace=True) degrades gracefully.
import sys as _sys, types as _types
try:
    import antenv.axon_hooks  # noqa: F401
except Exception:
    _m = _types.ModuleType("antenv.axon_hooks")
    _m.get_axon_ntff_profile_hook = lambda: None
    _sys.modules["antenv.axon_hooks"] = _m

import ml_dtypes
import concourse.bacc as bacc
import concourse.bass as bass
import concourse.mybir as mybir
import concourse.tile as tile
from concourse.bass_utils import run_bass_kernel_spmd

F32 = mybir.dt.float32
BF16 = mybir.dt.bfloat16
I16 = mybir.dt.int16
AF = mybir.ActivationFunctionType

N_NODES = 50000
N_EDGES = 800000
FEAT = 90
H = 128
STEPS = 4
N_GRAPHS = 100
N_CORES = 8
G_PAD = 16          # max graphs per core (padded)
G = 4               # dst groups (x128 nodes) per aggregation chunk
CHUNK = G * 128     # 512 nodes per chunk == GRU chunk
CAP = 16            # max rank blocks per gather call (nidx <= CAP*CHUNK)

LAST_RESULTS = {}   # stash for test.py (exec time etc)


# ----------------------------------------------------------------------------
# host-side layout
# ----------------------------------------------------------------------------

def _preprocess(x, edge_index, batch):
    batch = np.asarray(batch, np.int64)
    src = np.asarray(edge_index[0], np.int64)
    dst = np.asarray(edge_index[1], np.int64)

    counts = np.bincount(batch, minlength=N_GRAPHS)
    cum = np.concatenate([[0], np.cumsum(counts)])  # [101]
    targets = [round(N_NODES * c / N_CORES) for c in range(N_CORES + 1)]
    gsplit = [0]
    for c in range(1, N_CORES):
        g = int(np.argmin(np.abs(cum - targets[c])))
        g = max(g, gsplit[-1])
        gsplit.append(g)
    gsplit.append(N_GRAPHS)
    bounds = np.array([cum[g] for g in gsplit], np.int64)  # node bounds per core
    n_c = np.diff(bounds)
    assert (n_c > 0).all()
    n_graphs_c = np.diff(np.array(gsplit))
    assert n_graphs_c.max() <= G_PAD

    # n_pad: 512 multiple with a spare block holding the zero tile
    n_pad = int((np.ceil(n_c.max() / 512) + 1) * 512)
    table_rows = N_CORES * n_pad
    table_split = table_rows // 2
    assert table_split <= 32767
    n_chunks = int(np.ceil(n_c.max() / CHUNK))
    assert n_chunks * CHUNK <= n_pad - 128

    core_of = np.searchsorted(bounds, np.arange(N_NODES), side="right") - 1
    src_core = core_of[src]
    dst_core = core_of[dst]
    is_lo = src_core < (N_CORES // 2)

    # per-core permutation: descending by (max(lo,hi), min(lo,hi)) in-degree
    indeg_lo = np.bincount(dst[is_lo], minlength=N_NODES)
    indeg_hi = np.bincount(dst[~is_lo], minlength=N_NODES)
    newpos = np.zeros(N_NODES, np.int64)      # old global id -> permuted local id
    for c in range(N_CORES):
        sl = slice(bounds[c], bounds[c + 1])
        lo_c = indeg_lo[sl].astype(np.int64)
        hi_c = indeg_hi[sl].astype(np.int64)
        perm = np.lexsort((-np.minimum(lo_c, hi_c), -np.maximum(lo_c, hi_c)))
        inv = np.empty_like(perm)
        inv[perm] = np.arange(len(perm))
        newpos[sl] = inv

    # per (core, chunk, half) rank counts -> shared NR
    dstL = newpos[dst]
    ch_of = dstL // CHUNK
    cnt = np.zeros((N_CORES, n_chunks, 2), np.int64)
    # per-(dst,half) counts to compute chunk max
    pair = dst * 2 + (1 - is_lo.astype(np.int64))
    percnt = np.bincount(pair, minlength=2 * N_NODES).reshape(N_NODES, 2)
    for c in range(N_CORES):
        sl = slice(bounds[c], bounds[c + 1])
        pc = percnt[sl]
        chv = newpos[sl] // CHUNK
        for h in (0, 1):
            np.maximum.at(cnt[c, :, h], chv, pc[:, h])
    NR = cnt.max(axis=0)  # [n_chunks, 2]

    # round structure (shared): per (chunk, half) -> list of block counts
    rounds = []
    offs = np.zeros((n_chunks, 2), np.int64)  # start rank offset per (ch, h) in stream
    stream = []   # list of (ch, h, r0, nblk, off_idx)
    T = 0
    for ch in range(n_chunks):
        for h in (0, 1):
            nr = int(NR[ch, h])
            r0 = 0
            while r0 < nr:
                nb = min(CAP, nr - r0)
                stream.append((ch, h, r0, nb, T))
                T += nb * CHUNK
                r0 += nb
    assert T % 16 == 0

    # rank of each edge within (dst, half)
    order = np.lexsort((src, 1 - is_lo.astype(np.int64), dst))
    d_s = dst[order]
    h_s = (1 - is_lo.astype(np.int64))[order]
    key = d_s * 2 + h_s
    starts = np.flatnonzero(np.concatenate([[True], key[1:] != key[:-1]]))
    grp_id = np.cumsum(np.concatenate([[0], (key[1:] != key[:-1]).astype(np.int64)]))
    rank_s = np.arange(N_EDGES) - starts[grp_id]
    rank = np.empty(N_EDGES, np.int64)
    rank[order] = rank_s

    # idx value (relative to half base) for each edge
    srcL = newpos[src]
    rel = np.where(is_lo, src_core, src_core - N_CORES // 2) * n_pad + srcL
    assert rel.max() < table_split

    # offsets per (ch, h, r): map rank -> (round base, r_within)
    off_of = np.full((n_chunks, 2, int(NR.max()) + 1), -1, np.int64)
    for (ch, h, r0, nb, off) in stream:
        for rr in range(nb):
            off_of[ch, h, r0 + rr] = off + rr * CHUNK
    # edge stream position
    hh = (1 - is_lo.astype(np.int64))
    pos = off_of[ch_of, hh, rank] + (dstL % CHUNK)
    assert (pos >= 0).all()

    # build per-core idx arrays [16, T//16], default zero-row by slot
    zbase = n_pad - 128
    default_vals = (zbase + (np.arange(T) % 128)).astype(np.int16)
    # build stream value array per core then wrap
    per_core_idx = []
    for c in range(N_CORES):
        vals = default_vals.copy()
        m = dst_core == c
        vals[pos[m]] = rel[m].astype(np.int16)
        arr16 = np.zeros((16, T // 16), np.int16)
        p = np.arange(T)
        arr16[p % 16, p // 16] = vals
        per_core_idx.append(np.tile(arr16, (8, 1)))

    # per-core inputs
    nt = n_pad // 128
    per_core = []
    for c in range(N_CORES):
        nc_nodes = int(n_c[c])
        sl = slice(bounds[c], bounds[c + 1])
        inv = newpos[sl]
        xT = np.zeros((FEAT, n_pad), np.float32)
        xT[:, inv] = np.asarray(x[sl], np.float32).T
        gloc = (batch[sl] - gsplit[c]).astype(np.int64)
        gmat = np.zeros((128, nt * G_PAD), np.float32)
        gmat[inv % 128, (inv // 128) * G_PAD + gloc] = 1.0
        cc = counts[gsplit[c]:gsplit[c + 1]].astype(np.float32)
        invc = np.zeros((G_PAD, 1), np.float32)
        invc[:len(cc), 0] = 1.0 / np.maximum(cc, 1.0)
        per_core.append(dict(
            xT=xT,
            idx=per_core_idx[c],
            gmat=gmat,
            invc=invc,
        ))

    meta = dict(n_pad=n_pad, T=T, stream=stream, n_chunks=n_chunks, NR=NR,
                table_rows=table_rows, table_split=table_split,
                bounds=bounds, gsplit=gsplit, n_graphs_c=n_graphs_c)
    return per_core, meta


# ----------------------------------------------------------------------------
# device program
# ----------------------------------------------------------------------------

def _build(meta):
    n_pad = meta["n_pad"]
    T = meta["T"]
    stream = meta["stream"]
    n_chunks = meta["n_chunks"]
    table_rows = meta["table_rows"]
    table_split = meta["table_split"]
    nt = n_pad // 128          # 128-node tiles
    n_ch512 = n_pad // 512     # GRU chunks

    nc = bacc.Bacc("TRN2", target_bir_lowering=False, debug=False,
                   num_devices=N_CORES)

    d_xT = nc.dram_tensor("xT", [FEAT, n_pad], F32, kind="ExternalInput")
    d_idx = nc.dram_tensor("idx", [128, T // 16], I16, kind="ExternalInput")
    d_gmat = nc.dram_tensor("gmat", [128, nt * G_PAD], F32, kind="ExternalInput")
    d_invc = nc.dram_tensor("invc", [G_PAD, 1], F32, kind="ExternalInput")
    d_ident = nc.dram_tensor("ident", [128, 128], F32, kind="ExternalInput")
    d_wemb = nc.dram_tensor("wemb", [FEAT, H], F32, kind="ExternalInput")
    d_wmsg = nc.dram_tensor("wmsg", [STEPS, H, H], F32, kind="ExternalInput")
    d_wih = nc.dram_tensor("wih", [H, 3 * H], F32, kind="ExternalInput")
    d_whh = nc.dram_tensor("whh", [H, 3 * H], F32, kind="ExternalInput")
    d_bihT = nc.dram_tensor("bihT", [H, 3], F32, kind="ExternalInput")
    d_bhhT = nc.dram_tensor("bhhT", [H, 3], F32, kind="ExternalInput")
    d_w1 = nc.dram_tensor("w1", [H, H], F32, kind="ExternalInput")
    d_b1 = nc.dram_tensor("b1", [H, 1], F32, kind="ExternalInput")
    d_w2 = nc.dram_tensor("w2", [H, 1], F32, kind="ExternalInput")
    d_b2 = nc.dram_tensor("b2", [1, 1], F32, kind="ExternalInput")
    d_out = nc.dram_tensor("out", [1, G_PAD], F32, kind="ExternalOutput")

    with tile.TileContext(nc) as tc:
        with (
            tc.tile_pool(name="persist", bufs=1) as P,
            tc.tile_pool(name="dram", bufs=1, space="DRAM") as DR,
            tc.tile_pool(name="stg", bufs=2) as STG,
            tc.tile_pool(name="epool", bufs=2) as EP,
            tc.tile_pool(name="accpool", bufs=2) as ACC,
            tc.tile_pool(name="mpool", bufs=4) as MP,
            tc.tile_pool(name="gpool", bufs=2) as GP,
            tc.tile_pool(name="ps_m", bufs=2, space="PSUM") as PS_M,
            tc.tile_pool(name="ps_gru", bufs=4, space="PSUM") as PS_GRU,
        ):
            shard_bufs = [DR.tile([n_pad, H], BF16, name=f"m_shard{s_}")
                          for s_ in range(2)]
            table_bufs = [DR.tile([table_rows, H], BF16, addr_space="Shared",
                                  name=f"m_table{s_}") for s_ in range(STEPS)]

            # ---------------- constants / weights ----------------
            ix_all = P.tile([128, T // 16], I16, name="ix_all")
            nc.sync.dma_start(out=ix_all[:], in_=d_idx[:, :])

            ident_f = STG.tile([128, 128], F32, name="ident_f", tag="stgf")
            nc.sync.dma_start(out=ident_f[:], in_=d_ident[:, :])
            ident_b = P.tile([128, 128], BF16, name="ident_b")
            nc.vector.tensor_copy(ident_b[:], ident_f[:])

            def load_bf(dram_ap, shape, name):
                tf = STG.tile(shape, F32, name=name + "_f", tag="stgf")
                nc.sync.dma_start(out=tf[:], in_=dram_ap)
                tb = P.tile(shape, BF16, name=name + "_b")
                nc.scalar.activation(tb[:], tf[:], AF.Copy)
                return tb

            wemb_b = load_bf(d_wemb[:, :], [FEAT, H], "wemb")
            wmsg_b = [load_bf(d_wmsg[s, :, :], [H, H], f"wmsg{s}")
                      for s in range(STEPS)]
            wih_b = load_bf(d_wih[:, :], [H, 3 * H], "wih")
            whh_b = load_bf(d_whh[:, :], [H, 3 * H], "whh")
            w1_b = load_bf(d_w1[:, :], [H, H], "w1")
            w2_b = load_bf(d_w2[:, :], [H, 1], "w2")

            bih = P.tile([H, 3], F32, name="bih")
            nc.sync.dma_start(out=bih[:], in_=d_bihT[:, :])
            bhh = P.tile([H, 3], F32, name="bhh")
            nc.sync.dma_start(out=bhh[:], in_=d_bhhT[:, :])
            bsum = P.tile([H, 3], F32, name="bsum")
            nc.vector.tensor_add(bsum[:], bih[:], bhh[:])
            b1t = P.tile([H, 1], F32, name="b1t")
            nc.sync.dma_start(out=b1t[:], in_=d_b1[:, :])
            b2t = P.tile([1, 1], F32, name="b2t")
            nc.sync.dma_start(out=b2t[:], in_=d_b2[:, :])
            invc_t = P.tile([G_PAD, 1], F32, name="invc_t")
            nc.sync.dma_start(out=invc_t[:], in_=d_invc[:, :])
            gmat_f = STG.tile([128, nt * G_PAD], F32, name="gmat_f", tag="stgf")
            nc.sync.dma_start(out=gmat_f[:], in_=d_gmat[:, :])
            gmat_b = P.tile([128, nt * G_PAD], BF16, name="gmat_b")
            nc.scalar.activation(gmat_b[:], gmat_f[:], AF.Copy)

            # state; pad columns (beyond the last real chunk) stay zero for
            # the whole kernel - the per-chunk GRU loop never writes them.
            hA = P.tile([128, n_pad], F32, name="hA")
            hB = P.tile([128, n_pad], F32, name="hB")
            if n_chunks * 512 < n_pad:
                nc.vector.memset(hB[:, n_chunks * 512:], 0.0)
            h_bf_parts = []
            for j in range(n_ch512):
                hb_t = P.tile([128, 512], BF16, name=f"h_bf{j}")
                h_bf_parts.append(hb_t)
            m_all = P.tile([128, n_pad], BF16, name="m_all")
            # zero tile (last 128 columns) stays zero for the whole kernel
            nc.vector.memset(m_all[:, (nt - 1) * 128:nt * 128], 0.0)
            aggT_parts = []
            for j in range(n_ch512):
                ap_t = P.tile([128, 512], BF16, name=f"aggT{j}")
                nc.vector.memset(ap_t[:], 0.0)
                aggT_parts.append(ap_t)

            # ---------------- embedding ----------------
            for j in range(n_ch512):
                sl = slice(j * 512, (j + 1) * 512)
                xT_f = STG.tile([FEAT, 512], F32, name="xT_f", tag="stgx")
                nc.sync.dma_start(out=xT_f[:], in_=d_xT[:, sl])
                xT_b = STG.tile([FEAT, 512], BF16, name="xT_b", tag="stgxb")
                nc.scalar.activation(xT_b[:], xT_f[:], AF.Copy)
                pe = PS_GRU.tile([128, 512], F32, name="pe_emb", tag="pgru")
                nc.tensor.matmul(pe[:], lhsT=wemb_b[:, :], rhs=xT_b[:, :],
                                 start=True, stop=True)
                nc.scalar.activation(hA[:, sl], pe[:], AF.Relu)

            # chunk consumption plan: chunk -> list of (h, r0, nblk, off)
            chunk_rounds = [[] for _ in range(n_chunks)]
            for (ch, h, r0, nb, off) in stream:
                chunk_rounds[ch].append((h, r0, nb, off))

            # ---------------- message-passing steps ----------------
            for step in range(STEPS):
                h_cur = hA if step % 2 == 0 else hB
                h_nxt = hB if step % 2 == 0 else hA
                shard = shard_bufs[step % 2]
                table = table_bufs[step]

                for j in range(n_ch512):
                    nc.scalar.activation(h_bf_parts[j][:],
                                         h_cur[:, j * 512:(j + 1) * 512], AF.Copy)

                # messages (skip the zero tile)
                for t in range(nt - 1):
                    hb = h_bf_parts[t // 4][:, (t % 4) * 128:(t % 4 + 1) * 128]
                    pm = PS_M.tile([128, 128], F32, name="pm", tag="pmisc")
                    nc.tensor.matmul(pm[:], lhsT=hb,
                                     rhs=wmsg_b[step][:, :], start=True, stop=True)
                    nc.scalar.activation(m_all[:, t * 128:(t + 1) * 128],
                                         pm[:], AF.Copy)
                # node n = 128*t + p lives at m_all[p, t*128 : t*128+128]
                nc.sync.dma_start(
                    out=shard.rearrange("(a p) b -> p a b", p=128),
                    in_=m_all[:].rearrange("p (a b) -> p a b", b=128))

                nc.gpsimd.collective_compute(
                    "AllGather", mybir.AluOpType.bypass,
                    ins=[shard.opt()], outs=[table.opt()],
                    replica_groups=[list(range(N_CORES))],
                )
                tab_lo = table[0:table_split, :]
                tab_hi = table[table_split:table_rows, :]

                # aggregation: rank-gather (node-major) + DVE rank accumulate
                # E[q, i, f] = m[src of edge (rank r, group g, slot q)][f]
                # with i = r*G + g; per rank r, E[:, r*G:(r+1)*G, :] is a
                # contiguous [128, 512] slab summed into acc.
                for ch in range(n_chunks):
                    rounds = chunk_rounds[ch]
                    if not rounds:
                        continue
                    acc = ACC.tile([128, CHUNK], F32, name="acc", tag="acc")
                    done = 0
                    for (h, r0, nb, off) in rounds:
                        nidx = nb * CHUNK
                        E = EP.tile([128, CAP * G, H], BF16, name="E", tag="E")
                        tab = tab_lo if h == 0 else tab_hi
                        nc.gpsimd.dma_gather(
                            E[:, :nb * G, :], tab,
                            ix_all[:, off // 16:(off + nidx) // 16],
                            nidx, nidx, H, transpose=False,
                            single_packet=False)
                        for k in range(nb):
                            sl_e = E[:, k * G:(k + 1) * G, :]
                            if done == 0:
                                nc.vector.tensor_copy(acc[:], sl_e)
                            else:
                                nc.vector.tensor_add(acc[:], acc[:], sl_e)
                            done += 1
                    # transpose acc (node-major) into aggT (feature-major)
                    for g in range(G):
                        anm = MP.tile([128, 128], BF16, name="anm", tag="mp")
                        nc.scalar.activation(anm[:],
                                             acc[:, g * 128:(g + 1) * 128],
                                             AF.Copy)
                        ptr = PS_M.tile([128, 128], BF16, name="ptr", tag="pmisc")
                        nc.tensor.transpose(ptr[:], anm[:], ident_b[:])
                        nc.scalar.activation(
                            aggT_parts[ch][:, g * 128:(g + 1) * 128],
                            ptr[:], AF.Copy)

                    # GRU for this 512-node chunk (aggT chunk == GRU chunk)
                    j = ch
                    sl = slice(j * 512, (j + 1) * 512)
                    aT = aggT_parts[j]
                    p_r = PS_GRU.tile([128, 512], F32, name="p_r", tag="pgru")
                    nc.tensor.matmul(p_r[:], lhsT=wih_b[:, 0:128], rhs=aT[:, :],
                                     start=True, stop=False)
                    nc.tensor.matmul(p_r[:], lhsT=whh_b[:, 0:128],
                                     rhs=h_bf_parts[j][:, :],
                                     start=False, stop=True)
                    p_z = PS_GRU.tile([128, 512], F32, name="p_z", tag="pgru")
                    nc.tensor.matmul(p_z[:], lhsT=wih_b[:, 128:256], rhs=aT[:, :],
                                     start=True, stop=False)
                    nc.tensor.matmul(p_z[:], lhsT=whh_b[:, 128:256],
                                     rhs=h_bf_parts[j][:, :],
                                     start=False, stop=True)
                    p_xn = PS_GRU.tile([128, 512], F32, name="p_xn", tag="pgru")
                    nc.tensor.matmul(p_xn[:], lhsT=wih_b[:, 256:384], rhs=aT[:, :],
                                     start=True, stop=True)
                    p_hn = PS_GRU.tile([128, 512], F32, name="p_hn", tag="pgru")
                    nc.tensor.matmul(p_hn[:], lhsT=whh_b[:, 256:384],
                                     rhs=h_bf_parts[j][:, :],
                                     start=True, stop=True)
                    r_t = GP.tile([128, 512], BF16, name="r_t")
                    nc.scalar.activation(r_t[:], p_r[:], AF.Sigmoid, bias=bsum[:, 0:1])
                    z_t = GP.tile([128, 512], BF16, name="z_t")
                    nc.scalar.activation(z_t[:], p_z[:], AF.Sigmoid, bias=bsum[:, 1:2])
                    hn_t = GP.tile([128, 512], BF16, name="hn_t")
                    nc.scalar.activation(hn_t[:], p_hn[:], AF.Identity,
                                         bias=bhh[:, 2:3])
                    t1 = GP.tile([128, 512], BF16, name="t1")
                    nc.vector.tensor_mul(t1[:], r_t[:], hn_t[:])
                    u_t = GP.tile([128, 512], F32, name="u_t")
                    nc.vector.tensor_add(u_t[:], t1[:], p_xn[:])
                    n_t = GP.tile([128, 512], F32, name="n_t")
                    nc.scalar.activation(n_t[:], u_t[:], AF.Tanh, bias=bih[:, 2:3])
                    d_t = GP.tile([128, 512], F32, name="d_t")
                    nc.vector.tensor_sub(d_t[:], h_cur[:, sl], n_t[:])
                    e_t = GP.tile([128, 512], F32, name="e_t")
                    nc.vector.tensor_mul(e_t[:], z_t[:], d_t[:])
                    nc.vector.tensor_add(h_nxt[:, sl], n_t[:], e_t[:])

            # ---------------- readout ----------------
            h_fin = hA if STEPS % 2 == 0 else hB
            for j in range(n_ch512):
                nc.scalar.activation(h_bf_parts[j][:],
                                     h_fin[:, j * 512:(j + 1) * 512], AF.Relu)
            pp = PS_GRU.tile([G_PAD, 128], F32, name="pp", tag="pgru")
            for t in range(nt):
                hb = h_bf_parts[t // 4][:, (t % 4) * 128:(t % 4 + 1) * 128]
                ptr2 = PS_M.tile([128, 128], BF16, name="ptr2", tag="pmisc")
                nc.tensor.transpose(ptr2[:], hb, ident_b[:])
                hnm = MP.tile([128, 128], BF16, name="hnm", tag="mp")
                nc.scalar.activation(hnm[:], ptr2[:], AF.Copy)
                nc.tensor.matmul(pp[:], lhsT=gmat_b[:, t * G_PAD:(t + 1) * G_PAD],
                                 rhs=hnm[:], start=(t == 0), stop=(t == nt - 1))
            pooled = P.tile([G_PAD, 128], BF16, name="pooled")
            nc.vector.tensor_scalar(pooled[:], pp[:], invc_t[:], None,
                                    mybir.AluOpType.mult)
            ppt = PS_M.tile([128, G_PAD], BF16, name="ppt", tag="pmisc")
            nc.tensor.transpose(ppt[:], pooled[:], ident_b[0:G_PAD, 0:G_PAD])
            pooledT = P.tile([128, G_PAD], BF16, name="pooledT")
            nc.scalar.activation(pooledT[:], ppt[:], AF.Copy)
            pz1 = PS_M.tile([128, G_PAD], F32, name="pz1", tag="pmisc")
            nc.tensor.matmul(pz1[:], lhsT=w1_b[:, :], rhs=pooledT[:],
                             start=True, stop=True)
            z1 = P.tile([128, G_PAD], BF16, name="z1")
            nc.scalar.activation(z1[:], pz1[:], AF.Relu, bias=b1t[:, 0:1])
            po = PS_M.tile([1, G_PAD], F32, name="po", tag="pmisc")
            nc.tensor.matmul(po[:], lhsT=w2_b[:, :], rhs=z1[:],
                             start=True, stop=True)
            esb = P.tile([1, G_PAD], F32, name="esb")
            nc.scalar.activation(esb[:], po[:], AF.Exp, bias=b2t[:, 0:1])
            osb = P.tile([1, G_PAD], F32, name="osb")
            nc.scalar.activation(osb[:], esb[:], AF.Ln, bias=1.0)
            nc.sync.dma_start(out=d_out[:, :], in_=osb[:])

    nc.compile()
    return nc


# ----------------------------------------------------------------------------
# entry point
# ----------------------------------------------------------------------------

def kernel(x, edge_index, batch, W_emb, W_msg, W_ih, W_hh, b_ih, b_hh,
           W1, b1, W2, b2):
    x = np.asarray(x, np.float32)
    per_core, meta = _preprocess(x, edge_index, batch)
    nc = _build(meta)

    shared = dict(
        ident=np.eye(128, dtype=np.float32),
        wemb=np.asarray(W_emb, np.float32),
        wmsg=np.asarray(W_msg, np.float32),
        wih=np.asarray(W_ih, np.float32),
        whh=np.asarray(W_hh, np.float32),
        bihT=np.ascontiguousarray(np.asarray(b_ih, np.float32).reshape(3, H).T),
        bhhT=np.ascontiguousarray(np.asarray(b_hh, np.float32).reshape(3, H).T),
        w1=np.asarray(W1, np.float32),
        b1=np.asarray(b1, np.float32).reshape(H, 1),
        w2=np.asarray(W2, np.float32),
        b2=np.asarray(b2, np.float32).reshape(1, 1),
    )
    in_maps = []
    for c in range(N_CORES):
        m = dict(shared)
        m["xT"] = per_core[c]["xT"]
        m["idx"] = per_core[c]["idx"]
        m["gmat"] = per_core[c]["gmat"]
        m["invc"] = per_core[c]["invc"]
        in_maps.append(m)

    trace = bool(int(os.environ.get("KERNEL_TRACE", "0")))
    res = run_bass_kernel_spmd(nc, in_maps, list(range(N_CORES)), trace=trace)
    LAST_RESULTS["exec_time_ns"] = res.exec_time_ns
    LAST_RESULTS["profile_json"] = res.profile_json
    LAST_RESULTS["nc"] = nc
    LAST_RESULTS["in_maps"] = in_maps

    out = np.zeros((N_GRAPHS,), np.float32)
    gsplit = meta["gsplit"]
    for c in range(N_CORES):
        ng = gsplit[c + 1] - gsplit[c]
        out[gsplit[c]:gsplit[c + 1]] = res.results[c]["out"][0, :ng]
    return out


# revision 16
# speedup vs baseline: 2.3282x; 2.3282x over previous
"""Trainium2 Bass kernel for BondingGraphGNN (gnn_message_passing).

Model (see reference):
  h = relu(x @ W_emb)
  4x: m = h @ W_msg[i]; agg = scatter_add(m[src] -> dst); h = GRU(agg, h)
  h = relu(h); pooled = segment_mean(h, batch); out = softplus(relu(pooled@W1+b1)@W2+b2)

Distribution: graph-aligned node sharding across 8 cores. Per step each core
computes messages for its nodes (bf16), AllGathers the full node-message
table to DRAM, then aggregates incoming edges with a RANK-GATHER layout:
nodes are permuted so each 512-node chunk has near-uniform in-degree per
table half; the r-th incoming edge of every slot is fetched in one
transposed dma_gather block [feat, 512], and rank blocks are summed by
identity-matmul accumulation in PSUM (f32). Slots with fewer edges point at
a guaranteed-zero table row. No per-edge one-hot matrices are needed.

Host side does only data layout (shard/sort/pad/index building) - all float
math runs on device.
"""

import os
import numpy as np

# the trimmed axon package in some containers lacks the NTFF profile hook
# module; stub it so run_bass_kernel_spmd(trace=True) degrades gracefully.
import sys as _sys, types as _types
try:
    import antenv.axon_hooks  # noqa: F401
except Exception:
    _m = _types.ModuleType("antenv.axon_hooks")
    _m.get_axon_ntff_profile_hook = lambda: None
    _sys.modules["antenv.axon_hooks"] = _m

import ml_dtypes
import concourse.bacc as bacc
import concourse.bass as bass
import concourse.mybir as mybir
import concourse.tile as tile
from concourse.bass_utils import run_bass_kernel_spmd

F32 = mybir.dt.float32
BF16 = mybir.dt.bfloat16
I16 = mybir.dt.int16
AF = mybir.ActivationFunctionType

N_NODES = 50000
N_EDGES = 800000
FEAT = 90
H = 128
STEPS = 4
N_GRAPHS = 100
N_CORES = 8
G_PAD = 16          # max graphs per core (padded)
G = 4               # dst groups (x128 nodes) per aggregation chunk
CHUNK = G * 128     # 512 nodes per chunk == GRU chunk
CAP = 8             # max rank blocks per gather call (nidx <= CAP*CHUNK)
N_QUEUES = 4        # SWDGE queues; gathers on distinct queues overlap gen

LAST_RESULTS = {}   # stash for test.py (exec time etc)


# ----------------------------------------------------------------------------
# host-side layout
# ----------------------------------------------------------------------------

def _preprocess(x, edge_index, batch):
    batch = np.asarray(batch, np.int64)
    src = np.asarray(edge_index[0], np.int64)
    dst = np.asarray(edge_index[1], np.int64)

    counts = np.bincount(batch, minlength=N_GRAPHS)
    cum = np.concatenate([[0], np.cumsum(counts)])  # [101]
    targets = [round(N_NODES * c / N_CORES) for c in range(N_CORES + 1)]
    gsplit = [0]
    for c in range(1, N_CORES):
        g = int(np.argmin(np.abs(cum - targets[c])))
        g = max(g, gsplit[-1])
        gsplit.append(g)
    gsplit.append(N_GRAPHS)
    bounds = np.array([cum[g] for g in gsplit], np.int64)  # node bounds per core
    n_c = np.diff(bounds)
    assert (n_c > 0).all()
    n_graphs_c = np.diff(np.array(gsplit))
    assert n_graphs_c.max() <= G_PAD

    # n_pad: 512 multiple with a spare block holding the zero tile
    n_pad = int((np.ceil(n_c.max() / 512) + 1) * 512)
    table_rows = N_CORES * n_pad
    table_split = table_rows // 2
    assert table_split <= 32767
    n_chunks = int(np.ceil(n_c.max() / CHUNK))
    assert n_chunks * CHUNK <= n_pad - 128

    core_of = np.searchsorted(bounds, np.arange(N_NODES), side="right") - 1
    src_core = core_of[src]
    dst_core = core_of[dst]
    is_lo = src_core < (N_CORES // 2)

    # per-core permutation: descending by (max(lo,hi), min(lo,hi)) in-degree
    indeg_lo = np.bincount(dst[is_lo], minlength=N_NODES)
    indeg_hi = np.bincount(dst[~is_lo], minlength=N_NODES)
    newpos = np.zeros(N_NODES, np.int64)      # old global id -> permuted local id
    for c in range(N_CORES):
        sl = slice(bounds[c], bounds[c + 1])
        lo_c = indeg_lo[sl].astype(np.int64)
        hi_c = indeg_hi[sl].astype(np.int64)
        perm = np.lexsort((-np.minimum(lo_c, hi_c), -np.maximum(lo_c, hi_c)))
        inv = np.empty_like(perm)
        inv[perm] = np.arange(len(perm))
        newpos[sl] = inv

    # per (core, chunk, half) rank counts -> shared NR
    dstL = newpos[dst]
    ch_of = dstL // CHUNK
    cnt = np.zeros((N_CORES, n_chunks, 2), np.int64)
    # per-(dst,half) counts to compute chunk max
    pair = dst * 2 + (1 - is_lo.astype(np.int64))
    percnt = np.bincount(pair, minlength=2 * N_NODES).reshape(N_NODES, 2)
    for c in range(N_CORES):
        sl = slice(bounds[c], bounds[c + 1])
        pc = percnt[sl]
        chv = newpos[sl] // CHUNK
        for h in (0, 1):
            np.maximum.at(cnt[c, :, h], chv, pc[:, h])
    NR = cnt.max(axis=0)  # [n_chunks, 2]

    # round structure (shared): per (chunk, half) -> list of block counts
    rounds = []
    offs = np.zeros((n_chunks, 2), np.int64)  # start rank offset per (ch, h) in stream
    stream = []   # list of (ch, h, r0, nblk, off_idx)
    T = 0
    for ch in range(n_chunks):
        for h in (0, 1):
            nr = int(NR[ch, h])
            r0 = 0
            while r0 < nr:
                nb = min(CAP, nr - r0)
                stream.append((ch, h, r0, nb, T))
                T += nb * CHUNK
                r0 += nb
    assert T % 16 == 0

    # rank of each edge within (dst, half)
    order = np.lexsort((src, 1 - is_lo.astype(np.int64), dst))
    d_s = dst[order]
    h_s = (1 - is_lo.astype(np.int64))[order]
    key = d_s * 2 + h_s
    starts = np.flatnonzero(np.concatenate([[True], key[1:] != key[:-1]]))
    grp_id = np.cumsum(np.concatenate([[0], (key[1:] != key[:-1]).astype(np.int64)]))
    rank_s = np.arange(N_EDGES) - starts[grp_id]
    rank = np.empty(N_EDGES, np.int64)
    rank[order] = rank_s

    # idx value (relative to half base) for each edge
    srcL = newpos[src]
    rel = np.where(is_lo, src_core, src_core - N_CORES // 2) * n_pad + srcL
    assert rel.max() < table_split

    # offsets per (ch, h, r): map rank -> (round base, r_within)
    off_of = np.full((n_chunks, 2, int(NR.max()) + 1), -1, np.int64)
    for (ch, h, r0, nb, off) in stream:
        for rr in range(nb):
            off_of[ch, h, r0 + rr] = off + rr * CHUNK
    # edge stream position
    hh = (1 - is_lo.astype(np.int64))
    pos = off_of[ch_of, hh, rank] + (dstL % CHUNK)
    assert (pos >= 0).all()

    # build per-core idx arrays [16, T//16], default zero-row by slot
    zbase = n_pad - 128
    default_vals = (zbase + (np.arange(T) % 128)).astype(np.int16)
    # build stream value array per core then wrap
    per_core_idx = []
    for c in range(N_CORES):
        vals = default_vals.copy()
        m = dst_core == c
        vals[pos[m]] = rel[m].astype(np.int16)
        arr16 = np.zeros((16, T // 16), np.int16)
        p = np.arange(T)
        arr16[p % 16, p // 16] = vals
        per_core_idx.append(np.tile(arr16, (8, 1)))

    # per-core inputs
    nt = n_pad // 128
    per_core = []
    for c in range(N_CORES):
        nc_nodes = int(n_c[c])
        sl = slice(bounds[c], bounds[c + 1])
        inv = newpos[sl]
        xT = np.zeros((FEAT, n_pad), np.float32)
        xT[:, inv] = np.asarray(x[sl], np.float32).T
        gloc = (batch[sl] - gsplit[c]).astype(np.int64)
        gmat = np.zeros((128, nt * G_PAD), np.float32)
        gmat[inv % 128, (inv // 128) * G_PAD + gloc] = 1.0
        cc = counts[gsplit[c]:gsplit[c + 1]].astype(np.float32)
        invc = np.zeros((G_PAD, 1), np.float32)
        invc[:len(cc), 0] = 1.0 / np.maximum(cc, 1.0)
        per_core.append(dict(
            xT=xT,
            idx=per_core_idx[c],
            gmat=gmat,
            invc=invc,
        ))

    meta = dict(n_pad=n_pad, T=T, stream=stream, n_chunks=n_chunks, NR=NR,
                table_rows=table_rows, table_split=table_split,
                bounds=bounds, gsplit=gsplit, n_graphs_c=n_graphs_c)
    return per_core, meta


# ----------------------------------------------------------------------------
# device program
# ----------------------------------------------------------------------------

def _build(meta):
    n_pad = meta["n_pad"]
    T = meta["T"]
    stream = meta["stream"]
    n_chunks = meta["n_chunks"]
    table_rows = meta["table_rows"]
    table_split = meta["table_split"]
    nt = n_pad // 128          # 128-node tiles
    n_ch512 = n_pad // 512     # GRU chunks

    nc = bacc.Bacc("TRN2", target_bir_lowering=False, debug=False,
                   num_devices=N_CORES, num_swdge_queues=N_QUEUES,
                   dynamic_dma_scratch_size=32768)

    d_xT = nc.dram_tensor("xT", [FEAT, n_pad], F32, kind="ExternalInput")
    d_idx = nc.dram_tensor("idx", [128, T // 16], I16, kind="ExternalInput")
    d_gmat = nc.dram_tensor("gmat", [128, nt * G_PAD], F32, kind="ExternalInput")
    d_invc = nc.dram_tensor("invc", [G_PAD, 1], F32, kind="ExternalInput")
    d_ident = nc.dram_tensor("ident", [128, 128], F32, kind="ExternalInput")
    d_wemb = nc.dram_tensor("wemb", [FEAT, H], F32, kind="ExternalInput")
    d_wmsg = nc.dram_tensor("wmsg", [STEPS, H, H], F32, kind="ExternalInput")
    d_wih = nc.dram_tensor("wih", [H, 3 * H], F32, kind="ExternalInput")
    d_whh = nc.dram_tensor("whh", [H, 3 * H], F32, kind="ExternalInput")
    d_bihT = nc.dram_tensor("bihT", [H, 3], F32, kind="ExternalInput")
    d_bhhT = nc.dram_tensor("bhhT", [H, 3], F32, kind="ExternalInput")
    d_w1 = nc.dram_tensor("w1", [H, H], F32, kind="ExternalInput")
    d_b1 = nc.dram_tensor("b1", [H, 1], F32, kind="ExternalInput")
    d_w2 = nc.dram_tensor("w2", [H, 1], F32, kind="ExternalInput")
    d_b2 = nc.dram_tensor("b2", [1, 1], F32, kind="ExternalInput")
    d_out = nc.dram_tensor("out", [1, G_PAD], F32, kind="ExternalOutput")

    with tile.TileContext(nc) as tc:
        with (
            tc.tile_pool(name="persist", bufs=1) as P,
            tc.tile_pool(name="dram", bufs=1, space="DRAM") as DR,
            tc.tile_pool(name="stg", bufs=2) as STG,
            tc.tile_pool(name="epool", bufs=6) as EP,
            tc.tile_pool(name="mpool", bufs=4) as MP,
            tc.tile_pool(name="gpool", bufs=2) as GP,
            tc.tile_pool(name="ps_m", bufs=2, space="PSUM") as PS_M,
            tc.tile_pool(name="ps_agg", bufs=2, space="PSUM") as PS_AGG,
            tc.tile_pool(name="ps_gru", bufs=4, space="PSUM") as PS_GRU,
        ):
            shard_bufs = [DR.tile([n_pad, H], BF16, name=f"m_shard{s_}")
                          for s_ in range(2)]
            table_bufs = [DR.tile([table_rows, H], BF16, addr_space="Shared",
                                  name=f"m_table{s_}") for s_ in range(STEPS)]

            # ---------------- constants / weights ----------------
            ix_all = P.tile([128, T // 16], I16, name="ix_all")
            nc.sync.dma_start(out=ix_all[:], in_=d_idx[:, :])

            ident_f = STG.tile([128, 128], F32, name="ident_f", tag="stgf")
            nc.sync.dma_start(out=ident_f[:], in_=d_ident[:, :])
            ident_b = P.tile([128, 128], BF16, name="ident_b")
            nc.vector.tensor_copy(ident_b[:], ident_f[:])

            def load_bf(dram_ap, shape, name):
                tf = STG.tile(shape, F32, name=name + "_f", tag="stgf")
                nc.sync.dma_start(out=tf[:], in_=dram_ap)
                tb = P.tile(shape, BF16, name=name + "_b")
                nc.scalar.activation(tb[:], tf[:], AF.Copy)
                return tb

            wemb_b = load_bf(d_wemb[:, :], [FEAT, H], "wemb")
            wmsg_b = [load_bf(d_wmsg[s, :, :], [H, H], f"wmsg{s}")
                      for s in range(STEPS)]
            wih_b = load_bf(d_wih[:, :], [H, 3 * H], "wih")
            whh_b = load_bf(d_whh[:, :], [H, 3 * H], "whh")
            w1_b = load_bf(d_w1[:, :], [H, H], "w1")
            w2_b = load_bf(d_w2[:, :], [H, 1], "w2")

            bih = P.tile([H, 3], F32, name="bih")
            nc.sync.dma_start(out=bih[:], in_=d_bihT[:, :])
            bhh = P.tile([H, 3], F32, name="bhh")
            nc.sync.dma_start(out=bhh[:], in_=d_bhhT[:, :])
            bsum = P.tile([H, 3], F32, name="bsum")
            nc.vector.tensor_add(bsum[:], bih[:], bhh[:])
            b1t = P.tile([H, 1], F32, name="b1t")
            nc.sync.dma_start(out=b1t[:], in_=d_b1[:, :])
            b2t = P.tile([1, 1], F32, name="b2t")
            nc.sync.dma_start(out=b2t[:], in_=d_b2[:, :])
            invc_t = P.tile([G_PAD, 1], F32, name="invc_t")
            nc.sync.dma_start(out=invc_t[:], in_=d_invc[:, :])
            gmat_f = STG.tile([128, nt * G_PAD], F32, name="gmat_f", tag="stgf")
            nc.sync.dma_start(out=gmat_f[:], in_=d_gmat[:, :])
            gmat_b = P.tile([128, nt * G_PAD], BF16, name="gmat_b")
            nc.scalar.activation(gmat_b[:], gmat_f[:], AF.Copy)

            # state; single h updated in place per chunk (reads of h_cur
            # happen before the in-place write, ordered by Tile). Pad
            # columns stay zero: embedding writes relu(0)=0 there and the
            # per-chunk GRU loop never touches them.
            hh = P.tile([128, n_pad], F32, name="hh")
            h_bf_parts = []
            for j in range(n_ch512):
                hb_t = P.tile([128, 512], BF16, name=f"h_bf{j}")
                h_bf_parts.append(hb_t)
            m_all = P.tile([128, n_pad], BF16, name="m_all")
            # zero tile (last 128 columns) stays zero for the whole kernel
            nc.vector.memset(m_all[:, (nt - 1) * 128:nt * 128], 0.0)
            aggT_parts = []
            for j in range(n_ch512):
                ap_t = P.tile([128, 512], BF16, name=f"aggT{j}")
                nc.vector.memset(ap_t[:], 0.0)
                aggT_parts.append(ap_t)

            # ---------------- embedding ----------------
            for j in range(n_ch512):
                sl = slice(j * 512, (j + 1) * 512)
                xT_f = STG.tile([FEAT, 512], F32, name="xT_f", tag="stgx")
                nc.sync.dma_start(out=xT_f[:], in_=d_xT[:, sl])
                xT_b = STG.tile([FEAT, 512], BF16, name="xT_b", tag="stgxb")
                nc.scalar.activation(xT_b[:], xT_f[:], AF.Copy)
                pe = PS_GRU.tile([128, 512], F32, name="pe_emb", tag="pgru")
                nc.tensor.matmul(pe[:], lhsT=wemb_b[:, :], rhs=xT_b[:, :],
                                 start=True, stop=True)
                nc.scalar.activation(hh[:, sl], pe[:], AF.Relu)

            # chunk consumption plan: chunk -> list of (h, r0, nblk, off)
            chunk_rounds = [[] for _ in range(n_chunks)]
            for (ch, h, r0, nb, off) in stream:
                chunk_rounds[ch].append((h, r0, nb, off))
            qctr = [0]  # global gather counter for queue rotation

            # ---------------- message-passing steps ----------------
            for step in range(STEPS):
                h_cur = hh
                h_nxt = hh
                shard = shard_bufs[step % 2]
                table = table_bufs[step]

                for j in range(n_ch512):
                    nc.scalar.activation(h_bf_parts[j][:],
                                         h_cur[:, j * 512:(j + 1) * 512], AF.Copy)

                # messages (skip the zero tile)
                for t in range(nt - 1):
                    hb = h_bf_parts[t // 4][:, (t % 4) * 128:(t % 4 + 1) * 128]
                    pm = PS_M.tile([128, 128], F32, name="pm", tag="pmisc")
                    nc.tensor.matmul(pm[:], lhsT=hb,
                                     rhs=wmsg_b[step][:, :], start=True, stop=True)
                    nc.scalar.activation(m_all[:, t * 128:(t + 1) * 128],
                                         pm[:], AF.Copy)
                # node n = 128*t + p lives at m_all[p, t*128 : t*128+128]
                nc.sync.dma_start(
                    out=shard.rearrange("(a p) b -> p a b", p=128),
                    in_=m_all[:].rearrange("p (a b) -> p a b", b=128))

                nc.gpsimd.collective_compute(
                    "AllGather", mybir.AluOpType.bypass,
                    ins=[shard.opt()], outs=[table.opt()],
                    replica_groups=[list(range(N_CORES))],
                )
                tab_lo = table[0:table_split, :]
                tab_hi = table[table_split:table_rows, :]

                # aggregation: transposed rank-gather + identity-matmul
                # accumulate in PSUM. E[f, pos] with pos = (r*G+g)*128+q;
                # each rank block E[:, r*CHUNK:(r+1)*CHUNK] is copied through
                # the PE array (lhsT=I) into the accumulating [128, 512] psum.
                # Gathers rotate across the 4 SWDGE queues so their Q7
                # descriptor generation overlaps (4 distinct core pairs).
                for ch in range(n_chunks):
                    rounds = chunk_rounds[ch]
                    if not rounds:
                        continue
                    pa = PS_AGG.tile([128, CHUNK], F32, name="pa", tag="pa")
                    tot_blocks = sum(r[2] for r in rounds)
                    done = 0
                    for (h, r0, nb, off) in rounds:
                        nidx = nb * CHUNK
                        E = EP.tile([128, CAP * CHUNK], BF16, name="E", tag="E")
                        tab = tab_lo if h == 0 else tab_hi
                        nc.gpsimd.dma_gather(
                            E[:, :nidx].rearrange("p (o i) -> p o i", o=1), tab,
                            ix_all[:, off // 16:(off + nidx) // 16],
                            nidx, nidx, H, transpose=True,
                            single_packet=False, queue_num=qctr[0] % N_QUEUES)
                        qctr[0] += 1
                        for k in range(nb):
                            nc.tensor.matmul(
                                pa[:], lhsT=ident_b[:],
                                rhs=E[:, k * CHUNK:(k + 1) * CHUNK],
                                start=(done == 0),
                                stop=(done == tot_blocks - 1))
                            done += 1
                    nc.scalar.activation(aggT_parts[ch][:], pa[:], AF.Copy)

                    # GRU for this 512-node chunk (aggT chunk == GRU chunk)
                    j = ch
                    sl = slice(j * 512, (j + 1) * 512)
                    aT = aggT_parts[j]
                    p_r = PS_GRU.tile([128, 512], F32, name="p_r", tag="pgru")
                    nc.tensor.matmul(p_r[:], lhsT=wih_b[:, 0:128], rhs=aT[:, :],
                                     start=True, stop=False)
                    nc.tensor.matmul(p_r[:], lhsT=whh_b[:, 0:128],
                                     rhs=h_bf_parts[j][:, :],
                                     start=False, stop=True)
                    p_z = PS_GRU.tile([128, 512], F32, name="p_z", tag="pgru")
                    nc.tensor.matmul(p_z[:], lhsT=wih_b[:, 128:256], rhs=aT[:, :],
                                     start=True, stop=False)
                    nc.tensor.matmul(p_z[:], lhsT=whh_b[:, 128:256],
                                     rhs=h_bf_parts[j][:, :],
                                     start=False, stop=True)
                    p_xn = PS_GRU.tile([128, 512], F32, name="p_xn", tag="pgru")
                    nc.tensor.matmul(p_xn[:], lhsT=wih_b[:, 256:384], rhs=aT[:, :],
                                     start=True, stop=True)
                    p_hn = PS_GRU.tile([128, 512], F32, name="p_hn", tag="pgru")
                    nc.tensor.matmul(p_hn[:], lhsT=whh_b[:, 256:384],
                                     rhs=h_bf_parts[j][:, :],
                                     start=True, stop=True)
                    r_t = GP.tile([128, 512], BF16, name="r_t")
                    nc.scalar.activation(r_t[:], p_r[:], AF.Sigmoid, bias=bsum[:, 0:1])
                    z_t = GP.tile([128, 512], BF16, name="z_t")
                    nc.scalar.activation(z_t[:], p_z[:], AF.Sigmoid, bias=bsum[:, 1:2])
                    hn_t = GP.tile([128, 512], BF16, name="hn_t")
                    nc.scalar.activation(hn_t[:], p_hn[:], AF.Identity,
                                         bias=bhh[:, 2:3])
                    t1 = GP.tile([128, 512], BF16, name="t1")
                    nc.vector.tensor_mul(t1[:], r_t[:], hn_t[:])
                    u_t = GP.tile([128, 512], F32, name="u_t")
                    nc.vector.tensor_add(u_t[:], t1[:], p_xn[:])
                    n_t = GP.tile([128, 512], F32, name="n_t")
                    nc.scalar.activation(n_t[:], u_t[:], AF.Tanh, bias=bih[:, 2:3])
                    d_t = GP.tile([128, 512], F32, name="d_t")
                    nc.vector.tensor_sub(d_t[:], h_cur[:, sl], n_t[:])
                    e_t = GP.tile([128, 512], F32, name="e_t")
                    nc.vector.tensor_mul(e_t[:], z_t[:], d_t[:])
                    nc.vector.tensor_add(h_nxt[:, sl], n_t[:], e_t[:])

            # ---------------- readout ----------------
            h_fin = hh
            for j in range(n_ch512):
                nc.scalar.activation(h_bf_parts[j][:],
                                     h_fin[:, j * 512:(j + 1) * 512], AF.Relu)
            pp = PS_GRU.tile([G_PAD, 128], F32, name="pp", tag="pgru")
            for t in range(nt):
                hb = h_bf_parts[t // 4][:, (t % 4) * 128:(t % 4 + 1) * 128]
                ptr2 = PS_M.tile([128, 128], BF16, name="ptr2", tag="pmisc")
                nc.tensor.transpose(ptr2[:], hb, ident_b[:])
                hnm = MP.tile([128, 128], BF16, name="hnm", tag="mp")
                nc.scalar.activation(hnm[:], ptr2[:], AF.Copy)
                nc.tensor.matmul(pp[:], lhsT=gmat_b[:, t * G_PAD:(t + 1) * G_PAD],
                                 rhs=hnm[:], start=(t == 0), stop=(t == nt - 1))
            pooled = P.tile([G_PAD, 128], BF16, name="pooled")
            nc.vector.tensor_scalar(pooled[:], pp[:], invc_t[:], None,
                                    mybir.AluOpType.mult)
            ppt = PS_M.tile([128, G_PAD], BF16, name="ppt", tag="pmisc")
            nc.tensor.transpose(ppt[:], pooled[:], ident_b[0:G_PAD, 0:G_PAD])
            pooledT = P.tile([128, G_PAD], BF16, name="pooledT")
            nc.scalar.activation(pooledT[:], ppt[:], AF.Copy)
            pz1 = PS_M.tile([128, G_PAD], F32, name="pz1", tag="pmisc")
            nc.tensor.matmul(pz1[:], lhsT=w1_b[:, :], rhs=pooledT[:],
                             start=True, stop=True)
            z1 = P.tile([128, G_PAD], BF16, name="z1")
            nc.scalar.activation(z1[:], pz1[:], AF.Relu, bias=b1t[:, 0:1])
            po = PS_M.tile([1, G_PAD], F32, name="po", tag="pmisc")
            nc.tensor.matmul(po[:], lhsT=w2_b[:, :], rhs=z1[:],
                             start=True, stop=True)
            esb = P.tile([1, G_PAD], F32, name="esb")
            nc.scalar.activation(esb[:], po[:], AF.Exp, bias=b2t[:, 0:1])
            osb = P.tile([1, G_PAD], F32, name="osb")
            nc.scalar.activation(osb[:], esb[:], AF.Ln, bias=1.0)
            nc.sync.dma_start(out=d_out[:, :], in_=osb[:])

    nc.compile()
    return nc


# ----------------------------------------------------------------------------
# entry point
# ----------------------------------------------------------------------------

def kernel(x, edge_index, batch, W_emb, W_msg, W_ih, W_hh, b_ih, b_hh,
           W1, b1, W2, b2):
    x = np.asarray(x, np.float32)
    per_core, meta = _preprocess(x, edge_index, batch)
    nc = _build(meta)

    shared = dict(
        ident=np.eye(128, dtype=np.float32),
        wemb=np.asarray(W_emb, np.float32),
        wmsg=np.asarray(W_msg, np.float32),
        wih=np.asarray(W_ih, np.float32),
        whh=np.asarray(W_hh, np.float32),
        bihT=np.ascontiguousarray(np.asarray(b_ih, np.float32).reshape(3, H).T),
        bhhT=np.ascontiguousarray(np.asarray(b_hh, np.float32).reshape(3, H).T),
        w1=np.asarray(W1, np.float32),
        b1=np.asarray(b1, np.float32).reshape(H, 1),
        w2=np.asarray(W2, np.float32),
        b2=np.asarray(b2, np.float32).reshape(1, 1),
    )
    in_maps = []
    for c in range(N_CORES):
        m = dict(shared)
        m["xT"] = per_core[c]["xT"]
        m["idx"] = per_core[c]["idx"]
        m["gmat"] = per_core[c]["gmat"]
        m["invc"] = per_core[c]["invc"]
        in_maps.append(m)

    trace = bool(int(os.environ.get("KERNEL_TRACE", "0")))
    res = run_bass_kernel_spmd(nc, in_maps, list(range(N_CORES)), trace=trace)
    LAST_RESULTS["exec_time_ns"] = res.exec_time_ns
    LAST_RESULTS["profile_json"] = res.profile_json
    LAST_RESULTS["nc"] = nc
    LAST_RESULTS["in_maps"] = in_maps

    out = np.zeros((N_GRAPHS,), np.float32)
    gsplit = meta["gsplit"]
    for c in range(N_CORES):
        ng = gsplit[c + 1] - gsplit[c]
        out[gsplit[c]:gsplit[c + 1]] = res.results[c]["out"][0, :ng]
    return out


# revision 17
# speedup vs baseline: 3.1600x; 1.3573x over previous
"""Trainium2 Bass kernel for BondingGraphGNN (gnn_message_passing).

Model (see reference):
  h = relu(x @ W_emb)
  4x: m = h @ W_msg[i]; agg = scatter_add(m[src] -> dst); h = GRU(agg, h)
  h = relu(h); pooled = segment_mean(h, batch); out = softplus(relu(pooled@W1+b1)@W2+b2)

Distribution: graph-aligned node sharding across 8 cores. Per step each core
computes messages for its nodes (bf16), AllGathers the full node-message
table to DRAM, then aggregates incoming edges with a RANK-GATHER layout:
nodes are permuted so each 512-node chunk has near-uniform in-degree per
table half; the r-th incoming edge of every slot is fetched in one
transposed dma_gather block [feat, 512], and rank blocks are summed by
identity-matmul accumulation in PSUM (f32). Slots with fewer edges point at
a guaranteed-zero table row. No per-edge one-hot matrices are needed.

Host side does only data layout (shard/sort/pad/index building) - all float
math runs on device.
"""

import os
import numpy as np

# the trimmed axon package in some containers lacks the NTFF profile hook
# module; stub it so run_bass_kernel_spmd(trace=True) degrades gracefully.
import sys as _sys, types as _types
try:
    import antenv.axon_hooks  # noqa: F401
except Exception:
    _m = _types.ModuleType("antenv.axon_hooks")
    _m.get_axon_ntff_profile_hook = lambda: None
    _sys.modules["antenv.axon_hooks"] = _m

import ml_dtypes
import concourse.bacc as bacc
import concourse.bass as bass
import concourse.mybir as mybir
import concourse.tile as tile
from concourse.bass_utils import run_bass_kernel_spmd

F32 = mybir.dt.float32
BF16 = mybir.dt.bfloat16
I16 = mybir.dt.int16
AF = mybir.ActivationFunctionType

N_NODES = 50000
N_EDGES = 800000
FEAT = 90
H = 128
STEPS = 4
N_GRAPHS = 100
N_CORES = 8
G_PAD = 16          # max graphs per core (padded)
G = 4               # dst groups (x128 nodes) per aggregation chunk
CHUNK = G * 128     # 512 nodes per chunk == GRU chunk
CAP = 4             # max rank blocks per gather call (nidx <= CAP*CHUNK)
N_QUEUES = 4        # SWDGE queues; gathers on distinct queues overlap gen

LAST_RESULTS = {}   # stash for test.py (exec time etc)


# ----------------------------------------------------------------------------
# host-side layout
# ----------------------------------------------------------------------------

def _preprocess(x, edge_index, batch):
    batch = np.asarray(batch, np.int64)
    src = np.asarray(edge_index[0], np.int64)
    dst = np.asarray(edge_index[1], np.int64)

    counts = np.bincount(batch, minlength=N_GRAPHS)
    cum = np.concatenate([[0], np.cumsum(counts)])  # [101]
    targets = [round(N_NODES * c / N_CORES) for c in range(N_CORES + 1)]
    gsplit = [0]
    for c in range(1, N_CORES):
        g = int(np.argmin(np.abs(cum - targets[c])))
        g = max(g, gsplit[-1])
        gsplit.append(g)
    gsplit.append(N_GRAPHS)
    bounds = np.array([cum[g] for g in gsplit], np.int64)  # node bounds per core
    n_c = np.diff(bounds)
    assert (n_c > 0).all()
    n_graphs_c = np.diff(np.array(gsplit))
    assert n_graphs_c.max() <= G_PAD

    # n_pad: 512 multiple with a spare block holding the zero tile
    n_pad = int((np.ceil(n_c.max() / 512) + 1) * 512)
    table_rows = N_CORES * n_pad
    table_split = table_rows // 2
    assert table_split <= 32767
    n_chunks = int(np.ceil(n_c.max() / CHUNK))
    assert n_chunks * CHUNK <= n_pad - 128

    core_of = np.searchsorted(bounds, np.arange(N_NODES), side="right") - 1
    src_core = core_of[src]
    dst_core = core_of[dst]
    is_lo = src_core < (N_CORES // 2)

    # per-core permutation: descending by (max(lo,hi), min(lo,hi)) in-degree
    indeg_lo = np.bincount(dst[is_lo], minlength=N_NODES)
    indeg_hi = np.bincount(dst[~is_lo], minlength=N_NODES)
    newpos = np.zeros(N_NODES, np.int64)      # old global id -> permuted local id
    for c in range(N_CORES):
        sl = slice(bounds[c], bounds[c + 1])
        lo_c = indeg_lo[sl].astype(np.int64)
        hi_c = indeg_hi[sl].astype(np.int64)
        perm = np.lexsort((-np.minimum(lo_c, hi_c), -np.maximum(lo_c, hi_c)))
        inv = np.empty_like(perm)
        inv[perm] = np.arange(len(perm))
        newpos[sl] = inv

    # per (core, chunk, half) rank counts -> shared NR
    dstL = newpos[dst]
    ch_of = dstL // CHUNK
    cnt = np.zeros((N_CORES, n_chunks, 2), np.int64)
    # per-(dst,half) counts to compute chunk max
    pair = dst * 2 + (1 - is_lo.astype(np.int64))
    percnt = np.bincount(pair, minlength=2 * N_NODES).reshape(N_NODES, 2)
    for c in range(N_CORES):
        sl = slice(bounds[c], bounds[c + 1])
        pc = percnt[sl]
        chv = newpos[sl] // CHUNK
        for h in (0, 1):
            np.maximum.at(cnt[c, :, h], chv, pc[:, h])
    NR = cnt.max(axis=0)  # [n_chunks, 2]

    # round structure (shared): per (chunk, half) -> list of block counts
    rounds = []
    offs = np.zeros((n_chunks, 2), np.int64)  # start rank offset per (ch, h) in stream
    stream = []   # list of (ch, h, r0, nblk, off_idx)
    T = 0
    for ch in range(n_chunks):
        for h in (0, 1):
            nr = int(NR[ch, h])
            r0 = 0
            while r0 < nr:
                nb = min(CAP, nr - r0)
                stream.append((ch, h, r0, nb, T))
                T += nb * CHUNK
                r0 += nb
    assert T % 16 == 0

    # rank of each edge within (dst, half)
    order = np.lexsort((src, 1 - is_lo.astype(np.int64), dst))
    d_s = dst[order]
    h_s = (1 - is_lo.astype(np.int64))[order]
    key = d_s * 2 + h_s
    starts = np.flatnonzero(np.concatenate([[True], key[1:] != key[:-1]]))
    grp_id = np.cumsum(np.concatenate([[0], (key[1:] != key[:-1]).astype(np.int64)]))
    rank_s = np.arange(N_EDGES) - starts[grp_id]
    rank = np.empty(N_EDGES, np.int64)
    rank[order] = rank_s

    # idx value (relative to half base) for each edge
    srcL = newpos[src]
    rel = np.where(is_lo, src_core, src_core - N_CORES // 2) * n_pad + srcL
    assert rel.max() < table_split

    # offsets per (ch, h, r): map rank -> (round base, r_within)
    off_of = np.full((n_chunks, 2, int(NR.max()) + 1), -1, np.int64)
    for (ch, h, r0, nb, off) in stream:
        for rr in range(nb):
            off_of[ch, h, r0 + rr] = off + rr * CHUNK
    # edge stream position
    hh = (1 - is_lo.astype(np.int64))
    pos = off_of[ch_of, hh, rank] + (dstL % CHUNK)
    assert (pos >= 0).all()

    # build per-core idx arrays [16, T//16], default zero-row by slot
    zbase = n_pad - 128
    default_vals = (zbase + (np.arange(T) % 128)).astype(np.int16)
    # build stream value array per core then wrap
    per_core_idx = []
    for c in range(N_CORES):
        vals = default_vals.copy()
        m = dst_core == c
        vals[pos[m]] = rel[m].astype(np.int16)
        arr16 = np.zeros((16, T // 16), np.int16)
        p = np.arange(T)
        arr16[p % 16, p // 16] = vals
        per_core_idx.append(np.tile(arr16, (8, 1)))

    # per-core inputs
    nt = n_pad // 128
    per_core = []
    for c in range(N_CORES):
        nc_nodes = int(n_c[c])
        sl = slice(bounds[c], bounds[c + 1])
        inv = newpos[sl]
        xT = np.zeros((FEAT, n_pad), np.float32)
        xT[:, inv] = np.asarray(x[sl], np.float32).T
        gloc = (batch[sl] - gsplit[c]).astype(np.int64)
        gmat = np.zeros((128, nt * G_PAD), np.float32)
        gmat[inv % 128, (inv // 128) * G_PAD + gloc] = 1.0
        cc = counts[gsplit[c]:gsplit[c + 1]].astype(np.float32)
        invc = np.zeros((G_PAD, 1), np.float32)
        invc[:len(cc), 0] = 1.0 / np.maximum(cc, 1.0)
        per_core.append(dict(
            xT=xT,
            idx=per_core_idx[c],
            gmat=gmat,
            invc=invc,
        ))

    meta = dict(n_pad=n_pad, T=T, stream=stream, n_chunks=n_chunks, NR=NR,
                table_rows=table_rows, table_split=table_split,
                bounds=bounds, gsplit=gsplit, n_graphs_c=n_graphs_c)
    return per_core, meta


# ----------------------------------------------------------------------------
# device program
# ----------------------------------------------------------------------------

def _build(meta):
    n_pad = meta["n_pad"]
    T = meta["T"]
    stream = meta["stream"]
    n_chunks = meta["n_chunks"]
    table_rows = meta["table_rows"]
    table_split = meta["table_split"]
    nt = n_pad // 128          # 128-node tiles
    n_ch512 = n_pad // 512     # GRU chunks

    nc = bacc.Bacc("TRN2", target_bir_lowering=False, debug=False,
                   num_devices=N_CORES, num_swdge_queues=N_QUEUES,
                   dynamic_dma_scratch_size=32768)

    d_xT = nc.dram_tensor("xT", [FEAT, n_pad], F32, kind="ExternalInput")
    d_idx = nc.dram_tensor("idx", [128, T // 16], I16, kind="ExternalInput")
    d_gmat = nc.dram_tensor("gmat", [128, nt * G_PAD], F32, kind="ExternalInput")
    d_invc = nc.dram_tensor("invc", [G_PAD, 1], F32, kind="ExternalInput")
    d_ident = nc.dram_tensor("ident", [128, 128], F32, kind="ExternalInput")
    d_wemb = nc.dram_tensor("wemb", [FEAT, H], F32, kind="ExternalInput")
    d_wmsg = nc.dram_tensor("wmsg", [STEPS, H, H], F32, kind="ExternalInput")
    d_wih = nc.dram_tensor("wih", [H, 3 * H], F32, kind="ExternalInput")
    d_whh = nc.dram_tensor("whh", [H, 3 * H], F32, kind="ExternalInput")
    d_bihT = nc.dram_tensor("bihT", [H, 3], F32, kind="ExternalInput")
    d_bhhT = nc.dram_tensor("bhhT", [H, 3], F32, kind="ExternalInput")
    d_w1 = nc.dram_tensor("w1", [H, H], F32, kind="ExternalInput")
    d_b1 = nc.dram_tensor("b1", [H, 1], F32, kind="ExternalInput")
    d_w2 = nc.dram_tensor("w2", [H, 1], F32, kind="ExternalInput")
    d_b2 = nc.dram_tensor("b2", [1, 1], F32, kind="ExternalInput")
    d_out = nc.dram_tensor("out", [1, G_PAD], F32, kind="ExternalOutput")

    with tile.TileContext(nc) as tc:
        with (
            tc.tile_pool(name="persist", bufs=1) as P,
            tc.tile_pool(name="dram", bufs=1, space="DRAM") as DR,
            tc.tile_pool(name="stg", bufs=2) as STG,
            tc.tile_pool(name="epool", bufs=8) as EP,
            tc.tile_pool(name="mpool", bufs=4) as MP,
            tc.tile_pool(name="gpool", bufs=2) as GP,
            tc.tile_pool(name="ps_m", bufs=2, space="PSUM") as PS_M,
            tc.tile_pool(name="ps_agg", bufs=2, space="PSUM") as PS_AGG,
            tc.tile_pool(name="ps_gru", bufs=4, space="PSUM") as PS_GRU,
        ):
            shard_bufs = [DR.tile([n_pad, H], BF16, name=f"m_shard{s_}")
                          for s_ in range(2)]
            table_bufs = [DR.tile([table_rows, H], BF16, addr_space="Shared",
                                  name=f"m_table{s_}") for s_ in range(STEPS)]

            # ---------------- constants / weights ----------------
            ix_all = P.tile([128, T // 16], I16, name="ix_all")
            nc.sync.dma_start(out=ix_all[:], in_=d_idx[:, :])

            ident_f = STG.tile([128, 128], F32, name="ident_f", tag="stgf")
            nc.sync.dma_start(out=ident_f[:], in_=d_ident[:, :])
            ident_b = P.tile([128, 128], BF16, name="ident_b")
            nc.vector.tensor_copy(ident_b[:], ident_f[:])

            def load_bf(dram_ap, shape, name):
                tf = STG.tile(shape, F32, name=name + "_f", tag="stgf")
                nc.sync.dma_start(out=tf[:], in_=dram_ap)
                tb = P.tile(shape, BF16, name=name + "_b")
                nc.scalar.activation(tb[:], tf[:], AF.Copy)
                return tb

            wemb_b = load_bf(d_wemb[:, :], [FEAT, H], "wemb")
            wmsg_b = [load_bf(d_wmsg[s, :, :], [H, H], f"wmsg{s}")
                      for s in range(STEPS)]
            wih_b = load_bf(d_wih[:, :], [H, 3 * H], "wih")
            whh_b = load_bf(d_whh[:, :], [H, 3 * H], "whh")
            w1_b = load_bf(d_w1[:, :], [H, H], "w1")
            w2_b = load_bf(d_w2[:, :], [H, 1], "w2")

            bih = P.tile([H, 3], F32, name="bih")
            nc.sync.dma_start(out=bih[:], in_=d_bihT[:, :])
            bhh = P.tile([H, 3], F32, name="bhh")
            nc.sync.dma_start(out=bhh[:], in_=d_bhhT[:, :])
            bsum = P.tile([H, 3], F32, name="bsum")
            nc.vector.tensor_add(bsum[:], bih[:], bhh[:])
            b1t = P.tile([H, 1], F32, name="b1t")
            nc.sync.dma_start(out=b1t[:], in_=d_b1[:, :])
            b2t = P.tile([1, 1], F32, name="b2t")
            nc.sync.dma_start(out=b2t[:], in_=d_b2[:, :])
            invc_t = P.tile([G_PAD, 1], F32, name="invc_t")
            nc.sync.dma_start(out=invc_t[:], in_=d_invc[:, :])
            gmat_f = STG.tile([128, nt * G_PAD], F32, name="gmat_f", tag="stgf")
            nc.sync.dma_start(out=gmat_f[:], in_=d_gmat[:, :])
            gmat_b = P.tile([128, nt * G_PAD], BF16, name="gmat_b")
            nc.scalar.activation(gmat_b[:], gmat_f[:], AF.Copy)

            # state; single h updated in place per chunk (reads of h_cur
            # happen before the in-place write, ordered by Tile). Pad
            # columns stay zero: embedding writes relu(0)=0 there and the
            # per-chunk GRU loop never touches them.
            hh = P.tile([128, n_pad], F32, name="hh")
            h_bf_parts = []
            for j in range(n_ch512):
                hb_t = P.tile([128, 512], BF16, name=f"h_bf{j}")
                h_bf_parts.append(hb_t)
            m_all = P.tile([128, n_pad], BF16, name="m_all")
            # zero tile (last 128 columns) stays zero for the whole kernel
            nc.vector.memset(m_all[:, (nt - 1) * 128:nt * 128], 0.0)
            aggT_parts = []
            for j in range(n_ch512):
                ap_t = P.tile([128, 512], BF16, name=f"aggT{j}")
                nc.vector.memset(ap_t[:], 0.0)
                aggT_parts.append(ap_t)

            # ---------------- embedding ----------------
            for j in range(n_ch512):
                sl = slice(j * 512, (j + 1) * 512)
                xT_f = STG.tile([FEAT, 512], F32, name="xT_f", tag="stgx")
                nc.sync.dma_start(out=xT_f[:], in_=d_xT[:, sl])
                xT_b = STG.tile([FEAT, 512], BF16, name="xT_b", tag="stgxb")
                nc.scalar.activation(xT_b[:], xT_f[:], AF.Copy)
                pe = PS_GRU.tile([128, 512], F32, name="pe_emb", tag="pgru")
                nc.tensor.matmul(pe[:], lhsT=wemb_b[:, :], rhs=xT_b[:, :],
                                 start=True, stop=True)
                nc.scalar.activation(hh[:, sl], pe[:], AF.Relu)

            # chunk consumption plan: chunk -> list of (h, r0, nblk, off)
            chunk_rounds = [[] for _ in range(n_chunks)]
            for (ch, h, r0, nb, off) in stream:
                chunk_rounds[ch].append((h, r0, nb, off))
            qctr = [0]  # global gather counter for queue rotation

            # ---------------- message-passing steps ----------------
            for step in range(STEPS):
                h_cur = hh
                h_nxt = hh
                shard = shard_bufs[step % 2]
                table = table_bufs[step]

                for j in range(n_ch512):
                    nc.scalar.activation(h_bf_parts[j][:],
                                         h_cur[:, j * 512:(j + 1) * 512], AF.Copy)

                # messages (skip the zero tile)
                for t in range(nt - 1):
                    hb = h_bf_parts[t // 4][:, (t % 4) * 128:(t % 4 + 1) * 128]
                    pm = PS_M.tile([128, 128], F32, name="pm", tag="pmisc")
                    nc.tensor.matmul(pm[:], lhsT=hb,
                                     rhs=wmsg_b[step][:, :], start=True, stop=True)
                    nc.scalar.activation(m_all[:, t * 128:(t + 1) * 128],
                                         pm[:], AF.Copy)
                # node n = 128*t + p lives at m_all[p, t*128 : t*128+128]
                nc.sync.dma_start(
                    out=shard.rearrange("(a p) b -> p a b", p=128),
                    in_=m_all[:].rearrange("p (a b) -> p a b", b=128))

                nc.gpsimd.collective_compute(
                    "AllGather", mybir.AluOpType.bypass,
                    ins=[shard.opt()], outs=[table.opt()],
                    replica_groups=[list(range(N_CORES))],
                )
                tab_lo = table[0:table_split, :]
                tab_hi = table[table_split:table_rows, :]

                # aggregation: transposed rank-gather + identity-matmul
                # accumulate in PSUM. E[f, pos] with pos = (r*G+g)*128+q;
                # each rank block E[:, r*CHUNK:(r+1)*CHUNK] is copied through
                # the PE array (lhsT=I) into the accumulating [128, 512] psum.
                # Gathers rotate across the 4 SWDGE queues so their Q7
                # descriptor generation overlaps (4 distinct core pairs).
                for ch in range(n_chunks):
                    rounds = chunk_rounds[ch]
                    if not rounds:
                        continue
                    pa = PS_AGG.tile([128, CHUNK], F32, name="pa", tag="pa")
                    tot_blocks = sum(r[2] for r in rounds)
                    done = 0
                    for (h, r0, nb, off) in rounds:
                        nidx = nb * CHUNK
                        E = EP.tile([128, CAP * CHUNK], BF16, name="E", tag="E")
                        tab = tab_lo if h == 0 else tab_hi
                        nc.gpsimd.dma_gather(
                            E[:, :nidx].rearrange("p (o i) -> p o i", o=1), tab,
                            ix_all[:, off // 16:(off + nidx) // 16],
                            nidx, nidx, H, transpose=True,
                            single_packet=False, queue_num=qctr[0] % N_QUEUES)
                        qctr[0] += 1
                        for k in range(nb):
                            nc.tensor.matmul(
                                pa[:], lhsT=ident_b[:],
                                rhs=E[:, k * CHUNK:(k + 1) * CHUNK],
                                start=(done == 0),
                                stop=(done == tot_blocks - 1))
                            done += 1
                    nc.scalar.activation(aggT_parts[ch][:], pa[:], AF.Copy)

                    # GRU for this 512-node chunk (aggT chunk == GRU chunk)
                    j = ch
                    sl = slice(j * 512, (j + 1) * 512)
                    aT = aggT_parts[j]
                    p_r = PS_GRU.tile([128, 512], F32, name="p_r", tag="pgru")
                    nc.tensor.matmul(p_r[:], lhsT=wih_b[:, 0:128], rhs=aT[:, :],
                                     start=True, stop=False)
                    nc.tensor.matmul(p_r[:], lhsT=whh_b[:, 0:128],
                                     rhs=h_bf_parts[j][:, :],
                                     start=False, stop=True)
                    p_z = PS_GRU.tile([128, 512], F32, name="p_z", tag="pgru")
                    nc.tensor.matmul(p_z[:], lhsT=wih_b[:, 128:256], rhs=aT[:, :],
                                     start=True, stop=False)
                    nc.tensor.matmul(p_z[:], lhsT=whh_b[:, 128:256],
                                     rhs=h_bf_parts[j][:, :],
                                     start=False, stop=True)
                    p_xn = PS_GRU.tile([128, 512], F32, name="p_xn", tag="pgru")
                    nc.tensor.matmul(p_xn[:], lhsT=wih_b[:, 256:384], rhs=aT[:, :],
                                     start=True, stop=True)
                    p_hn = PS_GRU.tile([128, 512], F32, name="p_hn", tag="pgru")
                    nc.tensor.matmul(p_hn[:], lhsT=whh_b[:, 256:384],
                                     rhs=h_bf_parts[j][:, :],
                                     start=True, stop=True)
                    r_t = GP.tile([128, 512], BF16, name="r_t")
                    nc.scalar.activation(r_t[:], p_r[:], AF.Sigmoid, bias=bsum[:, 0:1])
                    z_t = GP.tile([128, 512], BF16, name="z_t")
                    nc.scalar.activation(z_t[:], p_z[:], AF.Sigmoid, bias=bsum[:, 1:2])
                    hn_t = GP.tile([128, 512], BF16, name="hn_t")
                    nc.scalar.activation(hn_t[:], p_hn[:], AF.Identity,
                                         bias=bhh[:, 2:3])
                    t1 = GP.tile([128, 512], BF16, name="t1")
                    nc.vector.tensor_mul(t1[:], r_t[:], hn_t[:])
                    u_t = GP.tile([128, 512], F32, name="u_t")
                    nc.vector.tensor_add(u_t[:], t1[:], p_xn[:])
                    n_t = GP.tile([128, 512], F32, name="n_t")
                    nc.scalar.activation(n_t[:], u_t[:], AF.Tanh, bias=bih[:, 2:3])
                    d_t = GP.tile([128, 512], F32, name="d_t")
                    nc.vector.tensor_sub(d_t[:], h_cur[:, sl], n_t[:])
                    e_t = GP.tile([128, 512], F32, name="e_t")
                    nc.vector.tensor_mul(e_t[:], z_t[:], d_t[:])
                    nc.vector.tensor_add(h_nxt[:, sl], n_t[:], e_t[:])

            # ---------------- readout ----------------
            h_fin = hh
            for j in range(n_ch512):
                nc.scalar.activation(h_bf_parts[j][:],
                                     h_fin[:, j * 512:(j + 1) * 512], AF.Relu)
            pp = PS_GRU.tile([G_PAD, 128], F32, name="pp", tag="pgru")
            for t in range(nt):
                hb = h_bf_parts[t // 4][:, (t % 4) * 128:(t % 4 + 1) * 128]
                ptr2 = PS_M.tile([128, 128], BF16, name="ptr2", tag="pmisc")
                nc.tensor.transpose(ptr2[:], hb, ident_b[:])
                hnm = MP.tile([128, 128], BF16, name="hnm", tag="mp")
                nc.scalar.activation(hnm[:], ptr2[:], AF.Copy)
                nc.tensor.matmul(pp[:], lhsT=gmat_b[:, t * G_PAD:(t + 1) * G_PAD],
                                 rhs=hnm[:], start=(t == 0), stop=(t == nt - 1))
            pooled = P.tile([G_PAD, 128], BF16, name="pooled")
            nc.vector.tensor_scalar(pooled[:], pp[:], invc_t[:], None,
                                    mybir.AluOpType.mult)
            ppt = PS_M.tile([128, G_PAD], BF16, name="ppt", tag="pmisc")
            nc.tensor.transpose(ppt[:], pooled[:], ident_b[0:G_PAD, 0:G_PAD])
            pooledT = P.tile([128, G_PAD], BF16, name="pooledT")
            nc.scalar.activation(pooledT[:], ppt[:], AF.Copy)
            pz1 = PS_M.tile([128, G_PAD], F32, name="pz1", tag="pmisc")
            nc.tensor.matmul(pz1[:], lhsT=w1_b[:, :], rhs=pooledT[:],
                             start=True, stop=True)
            z1 = P.tile([128, G_PAD], BF16, name="z1")
            nc.scalar.activation(z1[:], pz1[:], AF.Relu, bias=b1t[:, 0:1])
            po = PS_M.tile([1, G_PAD], F32, name="po", tag="pmisc")
            nc.tensor.matmul(po[:], lhsT=w2_b[:, :], rhs=z1[:],
                             start=True, stop=True)
            esb = P.tile([1, G_PAD], F32, name="esb")
            nc.scalar.activation(esb[:], po[:], AF.Exp, bias=b2t[:, 0:1])
            osb = P.tile([1, G_PAD], F32, name="osb")
            nc.scalar.activation(osb[:], esb[:], AF.Ln, bias=1.0)
            nc.sync.dma_start(out=d_out[:, :], in_=osb[:])

    nc.compile()
    return nc


# ----------------------------------------------------------------------------
# entry point
# ----------------------------------------------------------------------------

def kernel(x, edge_index, batch, W_emb, W_msg, W_ih, W_hh, b_ih, b_hh,
           W1, b1, W2, b2):
    x = np.asarray(x, np.float32)
    per_core, meta = _preprocess(x, edge_index, batch)
    nc = _build(meta)

    shared = dict(
        ident=np.eye(128, dtype=np.float32),
        wemb=np.asarray(W_emb, np.float32),
        wmsg=np.asarray(W_msg, np.float32),
        wih=np.asarray(W_ih, np.float32),
        whh=np.asarray(W_hh, np.float32),
        bihT=np.ascontiguousarray(np.asarray(b_ih, np.float32).reshape(3, H).T),
        bhhT=np.ascontiguousarray(np.asarray(b_hh, np.float32).reshape(3, H).T),
        w1=np.asarray(W1, np.float32),
        b1=np.asarray(b1, np.float32).reshape(H, 1),
        w2=np.asarray(W2, np.float32),
        b2=np.asarray(b2, np.float32).reshape(1, 1),
    )
    in_maps = []
    for c in range(N_CORES):
        m = dict(shared)
        m["xT"] = per_core[c]["xT"]
        m["idx"] = per_core[c]["idx"]
        m["gmat"] = per_core[c]["gmat"]
        m["invc"] = per_core[c]["invc"]
        in_maps.append(m)

    trace = bool(int(os.environ.get("KERNEL_TRACE", "0")))
    res = run_bass_kernel_spmd(nc, in_maps, list(range(N_CORES)), trace=trace)
    LAST_RESULTS["exec_time_ns"] = res.exec_time_ns
    LAST_RESULTS["profile_json"] = res.profile_json
    LAST_RESULTS["nc"] = nc
    LAST_RESULTS["in_maps"] = in_maps

    out = np.zeros((N_GRAPHS,), np.float32)
    gsplit = meta["gsplit"]
    for c in range(N_CORES):
        ng = gsplit[c + 1] - gsplit[c]
        out[gsplit[c]:gsplit[c + 1]] = res.results[c]["out"][0, :ng]
    return out
